# revision 3
# baseline (speedup 1.0000x reference)
"""KimiMoEGate on 8 Trainium2 NeuronCores.

Data-parallel over tokens: each core takes 1024 tokens, the full gate weight,
and produces (topk_idx, topk_weight) for its shard.

Device pipeline per 128-token block:
  - fp16 hi/lo split of x and w (x = hi + lo/256, exact to ~2^-22)
  - 3 full-speed PE passes: hi*hi -> psumA, hi*lo + lo*hi -> psumB
  - logits = psumA + psumB/256; sigmoid on ACT
  - grouped top-2 / top-4-group mask / top-8 experts via DVE max8/max_index/
    match_replace; weights recovered order-exactly via an 8x8 equality match.
"""
import sys
sys.path.insert(0, '/opt/trn_rl_repo')
import numpy as np
import concourse.bass as bass
from concourse import bacc
import concourse.mybir as mybir
from concourse.bass_utils import run_bass_kernel_spmd
from concourse.tile import TileContext

F32 = mybir.dt.float32
F16 = mybir.dt.float16
U32 = mybir.dt.uint32
I32 = mybir.dt.int32
AX = mybir.AxisListType
ALU = mybir.AluOpType
ACTF = mybir.ActivationFunctionType

T, H, E = 8192, 7168, 256
NCORES = 8
TPC = T // NCORES            # 1024 tokens per core
KT = H // 128                # 56 contraction tiles
NB = TPC // 128              # 8 blocks of 128 tokens
CH = 8                       # k-tiles per DMA/split chunk
NEG = -1e30

_cache = {}
TRACE = False          # set by test.py to capture an NTFF profile
LAST_RESULTS = None    # BassKernelResults from the most recent run


def _build():
    if "nc" in _cache:
        return _cache["nc"]
    nc = bacc.Bacc("TRN2", target_bir_lowering=False, debug=False,
                   num_devices=NCORES)
    xtb = nc.dram_tensor("xtb", [NB, KT, 128, 128], F32, kind="ExternalInput")
    wt = nc.dram_tensor("wt", [KT, 128, E], F32, kind="ExternalInput")
    bias = nc.dram_tensor("bias", [E], F32, kind="ExternalInput")
    o_idx = nc.dram_tensor("o_idx", [TPC, 8], I32, kind="ExternalOutput")
    o_w = nc.dram_tensor("o_w", [TPC, 8], F32, kind="ExternalOutput")

    with TileContext(nc) as tc:
        with (
            tc.tile_pool(name="wpool", bufs=1) as wpool,
            tc.tile_pool(name="xpool", bufs=2) as xpool,
            tc.tile_pool(name="stage", bufs=3) as stage,
            tc.tile_pool(name="small", bufs=2) as small,
            tc.tile_pool(name="ps", bufs=2, space="PSUM") as ps,
        ):
            # ---- weight prep (once): split w into f16 hi + lo*256 ----
            w_hi = wpool.tile([128, KT, E], F16)
            w_lo = wpool.tile([128, KT, E], F16)
            for k0 in range(0, KT, CH):
                ksl = slice(k0, k0 + CH)
                wc = stage.tile([128, CH, E], F32, tag="wstage")
                nc.sync.dma_start(wc[:], wt.rearrange("k p m -> p k m")[:, ksl])
                nc.scalar.activation(w_hi[:, ksl], wc[:], ACTF.Copy)
                lo32 = stage.tile([128, CH, E], F32, tag="wlo32")
                nc.vector.tensor_tensor(lo32[:], wc[:], w_hi[:, ksl],
                                        ALU.subtract)
                nc.vector.tensor_scalar(w_lo[:, ksl], lo32[:], 256.0, None,
                                        op0=ALU.mult)
            bias_rep = wpool.tile([128, E], F32)
            nc.sync.dma_start(bias_rep[:], bias[None, :].to_broadcast([128, E]))

            for b in range(NB):
                # ---- x split for this block ----
                x_hi = xpool.tile([128, KT, 128], F16, tag="xhi")
                x_lo = xpool.tile([128, KT, 128], F16, tag="xlo")
                for k0 in range(0, KT, CH):
                    ksl = slice(k0, k0 + CH)
                    xc = stage.tile([128, CH, 128], F32, tag="xstage")
                    nc.sync.dma_start(xc[:], xtb[b, ksl].rearrange("k p n -> p k n"))
                    nc.scalar.activation(x_hi[:, ksl], xc[:], ACTF.Copy)
                    lo32 = stage.tile([128, CH, 128], F32, tag="xlo32")
                    nc.vector.tensor_tensor(lo32[:], xc[:], x_hi[:, ksl],
                                            ALU.subtract)
                    nc.scalar.activation(x_lo[:, ksl], lo32[:], ACTF.Copy,
                                         scale=256.0)

                # ---- 3 GEMM passes ----
                psA = ps.tile([128, E], F32, tag="psA")
                psB = ps.tile([128, E], F32, tag="psB")
                for k in range(KT):
                    nc.tensor.matmul(psA[:], x_hi[:, k], w_hi[:, k],
                                     start=(k == 0), stop=(k == KT - 1))
                for k in range(KT):
                    nc.tensor.matmul(psB[:], x_hi[:, k], w_lo[:, k],
                                     start=(k == 0), stop=False)
                for k in range(KT):
                    nc.tensor.matmul(psB[:], x_lo[:, k], w_hi[:, k],
                                     start=False, stop=(k == KT - 1))

                # ---- epilogue: logits, sigmoid, routing ----
                logA = small.tile([128, E], F32, tag="logA")
                nc.scalar.activation(logA[:], psA[:], ACTF.Copy)
                logits = small.tile([128, E], F32, tag="logits")
                nc.vector.scalar_tensor_tensor(logits[:], psB[:], 1.0 / 256.0,
                                               logA[:], op0=ALU.mult, op1=ALU.add)
                s = small.tile([128, E], F32, tag="s")
                nc.scalar.activation(s[:], logits[:], ACTF.Sigmoid)
                sc = small.tile([128, E], F32, tag="sc")
                nc.vector.tensor_tensor(sc[:], s[:], bias_rep[:], ALU.add)

                scg = sc[:].rearrange("p (g e) -> p g e", g=8)
                gm = small.tile([128, 8], F32, tag="gm")
                nc.vector.tensor_reduce(gm[:], scg, AX.X, ALU.max)
                scr = small.tile([128, E], F32, tag="scr")
                nc.vector.match_replace(scr[:], gm[:], sc[:], NEG)
                gm2 = small.tile([128, 8], F32, tag="gm2")
                nc.vector.tensor_reduce(
                    gm2[:], scr[:].rearrange("p (g e) -> p g e", g=8),
                    AX.X, ALU.max)
                gsum = small.tile([128, 8], F32, tag="gsum")
                nc.vector.tensor_tensor(gsum[:], gm[:], gm2[:], ALU.add)
                g8 = small.tile([128, 8], F32, tag="g8")
                nc.vector.max(g8[:], gsum[:])
                gmask = small.tile([128, 8], F32, tag="gmask")
                nc.vector.tensor_scalar(gmask[:], gsum[:], g8[:, 3:4], None,
                                        op0=ALU.is_ge)
                tmp = small.tile([128, E], F32, tag="tmp")
                nc.vector.tensor_tensor(
                    tmp[:].rearrange("p (g e) -> p g e", g=8), scg,
                    gmask[:, :, None].to_broadcast([128, 8, 32]), ALU.mult)
                v8 = small.tile([128, 8], F32, tag="v8")
                nc.vector.max(v8[:], tmp[:])
                i8 = small.tile([128, 8], U32, tag="i8")
                nc.vector.max_index(i8[:], v8[:], tmp[:])

                marked = small.tile([128, E], F32, tag="marked")
                nc.vector.match_replace(marked[:], v8[:], tmp[:], NEG)
                possel = small.tile([128, E], F32, tag="possel")
                nc.vector.tensor_tensor(possel[:], tmp[:], marked[:],
                                        ALU.not_equal)
                s_sel = small.tile([128, E], F32, tag="s_sel")
                nc.vector.tensor_tensor(s_sel[:], s[:], possel[:], ALU.mult)
                w8s = small.tile([128, 8], F32, tag="w8s")
                nc.vector.max(w8s[:], s_sel[:])
                is8 = small.tile([128, 8], U32, tag="is8")
                nc.vector.max_index(is8[:], w8s[:], s_sel[:])

                eq = small.tile([128, 8, 8], F32, tag="eq")
                nc.vector.tensor_tensor(
                    eq[:],
                    is8[:, None, :].to_broadcast([128, 8, 8]),
                    i8[:, :, None].to_broadcast([128, 8, 8]),
                    ALU.is_equal)
                prod = small.tile([128, 8, 8], F32, tag="prod")
                nc.vector.tensor_tensor(
                    prod[:], eq[:],
                    w8s[:, None, :].to_broadcast([128, 8, 8]), ALU.mult)
                w8 = small.tile([128, 8], F32, tag="w8")
                nc.vector.tensor_reduce(w8[:], prod[:], AX.X, ALU.add)
                ssum = small.tile([128, 1], F32, tag="ssum")
                nc.vector.tensor_reduce(ssum[:], w8s[:], AX.X, ALU.add)
                rec = small.tile([128, 1], F32, tag="rec")
                nc.vector.reciprocal(rec[:], ssum[:])
                rec25 = small.tile([128, 1], F32, tag="rec25")
                nc.vector.tensor_scalar(rec25[:], rec[:], 2.5, None,
                                        op0=ALU.mult)
                wfin = small.tile([128, 8], F32, tag="wfin")
                nc.vector.tensor_scalar(wfin[:], w8[:], rec25[:], None,
                                        op0=ALU.mult)
                nc.sync.dma_start(o_w[b * 128:(b + 1) * 128], wfin[:])
                nc.sync.dma_start(o_idx[b * 128:(b + 1) * 128],
                                  i8[:].bitcast(I32))
    nc.compile()
    _cache["nc"] = nc
    return nc


def kernel(hidden_states, weight, e_score_correction_bias):
    nc = _build()
    x = np.asarray(hidden_states, dtype=np.float32)
    w = np.asarray(weight, dtype=np.float32)
    b = np.asarray(e_score_correction_bias, dtype=np.float32)

    # layout prep (host): transposed, k-tiled, block-contiguous
    wt = np.ascontiguousarray(w.T.reshape(KT, 128, E))
    in_maps = []
    for c in range(NCORES):
        xs = x[c * TPC:(c + 1) * TPC]                     # [1024, H]
        xt = xs.T.reshape(KT, 128, NB, 128)               # [k, p, b, n]
        xtb = np.ascontiguousarray(xt.transpose(2, 0, 1, 3))
        in_maps.append({"xtb": xtb, "wt": wt, "bias": b})

    global LAST_RESULTS
    br = run_bass_kernel_spmd(nc, in_maps, list(range(NCORES)), trace=TRACE)
    LAST_RESULTS = br
    res = br.results
    idx = np.concatenate([res[c]["o_idx"] for c in range(NCORES)], axis=0)
    wgt = np.concatenate([res[c]["o_w"] for c in range(NCORES)], axis=0)
    return idx.astype(np.int32), wgt.astype(np.float32)



# revision 5
# speedup vs baseline: 1.0532x; 1.0532x over previous
"""KimiMoEGate on 8 Trainium2 NeuronCores.

Data-parallel over tokens: each core takes 1024 tokens, the full gate weight,
and produces (topk_idx, topk_weight) for its shard.

v1.5: host-side fp16 hi/lo split (exact to ~2^-22) removes all on-device
elementwise split work, and the DMA layout gives each partition multi-KB
contiguous runs.  Device does 3 full-speed fp16 PE passes per 128-token
block (hi*hi -> psA, hi*lo + lo*hi -> psB, lo scaled by 2048), then the
DVE top-k routing epilogue:
  grouped top-2 / top-4-group mask / top-8 experts via max8/max_index/
  match_replace; weights recovered order-exactly via an 8x8 equality match.
"""
import sys
sys.path.insert(0, '/opt/trn_rl_repo')
import numpy as np
import concourse.bass as bass
from concourse import bacc
import concourse.mybir as mybir
from concourse.bass_utils import run_bass_kernel_spmd
from concourse.tile import TileContext

F32 = mybir.dt.float32
F16 = mybir.dt.float16
U32 = mybir.dt.uint32
I32 = mybir.dt.int32
AX = mybir.AxisListType
ALU = mybir.AluOpType
ACTF = mybir.ActivationFunctionType

T, H, E = 8192, 7168, 256
NCORES = 8
TPC = T // NCORES            # 1024 tokens per core
KT = H // 128                # 56 contraction tiles
NB = TPC // 128              # 8 blocks of 128 tokens
NC = 2                       # k-chunks per block (28 k-tiles each)
KC = KT // NC
LSCALE = 2048.0              # lo-part scale (2^11)
NEG = -1e30

_cache = {}
TRACE = False          # set by test.py to capture an NTFF profile
LAST_RESULTS = None    # BassKernelResults from the most recent run


def _build():
    if "nc" in _cache:
        return _cache["nc"]
    nc = bacc.Bacc("TRN2", target_bir_lowering=False, debug=False,
                   num_devices=NCORES)
    # host-split operands: [block, chunk, part, k-in-chunk, token]
    xh_d = nc.dram_tensor("xh", [NB, NC, 128, KC, 128], F16, kind="ExternalInput")
    xl_d = nc.dram_tensor("xl", [NB, NC, 128, KC, 128], F16, kind="ExternalInput")
    wh_d = nc.dram_tensor("wh", [NC, 128, KC, E], F16, kind="ExternalInput")
    wl_d = nc.dram_tensor("wl", [NC, 128, KC, E], F16, kind="ExternalInput")
    bias = nc.dram_tensor("bias", [E], F32, kind="ExternalInput")
    o_idx = nc.dram_tensor("o_idx", [TPC, 8], I32, kind="ExternalOutput")
    o_w = nc.dram_tensor("o_w", [TPC, 8], F32, kind="ExternalOutput")

    with TileContext(nc) as tc:
        with (
            tc.tile_pool(name="wpool", bufs=1) as wpool,
            tc.tile_pool(name="xpool", bufs=2) as xpool,
            tc.tile_pool(name="small", bufs=2) as small,
            tc.tile_pool(name="ps", bufs=2, space="PSUM") as ps,
        ):
            # resident weights, chunk-granular DMA so block 0 can start early
            w_hi = wpool.tile([128, KT, E], F16)
            w_lo = wpool.tile([128, KT, E], F16)
            for c in range(NC):
                ksl = slice(c * KC, (c + 1) * KC)
                nc.sync.dma_start(w_hi[:, ksl], wh_d[c])
                nc.sync.dma_start(w_lo[:, ksl], wl_d[c])
            bias_rep = wpool.tile([128, E], F32)
            nc.sync.dma_start(bias_rep[:], bias[None, :].to_broadcast([128, E]))

            for b in range(NB):
                xh = [xpool.tile([128, KC, 128], F16, tag=f"xh{c}",
                                 name=f"xh{c}_{b}") for c in range(NC)]
                xl = [xpool.tile([128, KC, 128], F16, tag=f"xl{c}",
                                 name=f"xl{c}_{b}") for c in range(NC)]
                for c in range(NC):
                    nc.sync.dma_start(xh[c][:], xh_d[b, c])
                for c in range(NC):
                    nc.sync.dma_start(xl[c][:], xl_d[b, c])

                psA = ps.tile([128, E], F32, tag="psA")
                psB = ps.tile([128, E], F32, tag="psB")
                # hi*hi and hi*lo consume xh chunk-locally; lo*hi then xl
                for c in range(NC):
                    for k in range(KC):
                        kg = c * KC + k
                        nc.tensor.matmul(psA[:], xh[c][:, k], w_hi[:, kg],
                                         start=(kg == 0), stop=(kg == KT - 1))
                    for k in range(KC):
                        kg = c * KC + k
                        nc.tensor.matmul(psB[:], xh[c][:, k], w_lo[:, kg],
                                         start=(kg == 0), stop=False)
                for c in range(NC):
                    for k in range(KC):
                        kg = c * KC + k
                        nc.tensor.matmul(psB[:], xl[c][:, k], w_hi[:, kg],
                                         start=False, stop=(kg == KT - 1))

                # ---- epilogue: logits, sigmoid, routing ----
                logA = small.tile([128, E], F32, tag="logA")
                nc.scalar.activation(logA[:], psA[:], ACTF.Copy)
                logits = small.tile([128, E], F32, tag="logits")
                nc.vector.scalar_tensor_tensor(logits[:], psB[:], 1.0 / LSCALE,
                                               logA[:], op0=ALU.mult, op1=ALU.add)
                s = small.tile([128, E], F32, tag="s")
                nc.scalar.activation(s[:], logits[:], ACTF.Sigmoid)
                sc = small.tile([128, E], F32, tag="sc")
                nc.vector.tensor_tensor(sc[:], s[:], bias_rep[:], ALU.add)

                scg = sc[:].rearrange("p (g e) -> p g e", g=8)
                gm = small.tile([128, 8], F32, tag="gm")
                nc.vector.tensor_reduce(gm[:], scg, AX.X, ALU.max)
                scr = small.tile([128, E], F32, tag="scr")
                nc.vector.match_replace(scr[:], gm[:], sc[:], NEG)
                gm2 = small.tile([128, 8], F32, tag="gm2")
                nc.vector.tensor_reduce(
                    gm2[:], scr[:].rearrange("p (g e) -> p g e", g=8),
                    AX.X, ALU.max)
                gsum = small.tile([128, 8], F32, tag="gsum")
                nc.vector.tensor_tensor(gsum[:], gm[:], gm2[:], ALU.add)
                g8 = small.tile([128, 8], F32, tag="g8")
                nc.vector.max(g8[:], gsum[:])
                gmask = small.tile([128, 8], F32, tag="gmask")
                nc.vector.tensor_scalar(gmask[:], gsum[:], g8[:, 3:4], None,
                                        op0=ALU.is_ge)
                tmp = small.tile([128, E], F32, tag="tmp")
                nc.vector.tensor_tensor(
                    tmp[:].rearrange("p (g e) -> p g e", g=8), scg,
                    gmask[:, :, None].to_broadcast([128, 8, 32]), ALU.mult)
                v8 = small.tile([128, 8], F32, tag="v8")
                nc.vector.max(v8[:], tmp[:])
                i8 = small.tile([128, 8], U32, tag="i8")
                nc.vector.max_index(i8[:], v8[:], tmp[:])

                marked = small.tile([128, E], F32, tag="marked")
                nc.vector.match_replace(marked[:], v8[:], tmp[:], NEG)
                possel = small.tile([128, E], F32, tag="possel")
                nc.vector.tensor_tensor(possel[:], tmp[:], marked[:],
                                        ALU.not_equal)
                s_sel = small.tile([128, E], F32, tag="s_sel")
                nc.vector.tensor_tensor(s_sel[:], s[:], possel[:], ALU.mult)
                w8s = small.tile([128, 8], F32, tag="w8s")
                nc.vector.max(w8s[:], s_sel[:])
                is8 = small.tile([128, 8], U32, tag="is8")
                nc.vector.max_index(is8[:], w8s[:], s_sel[:])

                eq = small.tile([128, 8, 8], F32, tag="eq")
                nc.vector.tensor_tensor(
                    eq[:],
                    is8[:, None, :].to_broadcast([128, 8, 8]),
                    i8[:, :, None].to_broadcast([128, 8, 8]),
                    ALU.is_equal)
                prod = small.tile([128, 8, 8], F32, tag="prod")
                nc.vector.tensor_tensor(
                    prod[:], eq[:],
                    w8s[:, None, :].to_broadcast([128, 8, 8]), ALU.mult)
                w8 = small.tile([128, 8], F32, tag="w8")
                nc.vector.tensor_reduce(w8[:], prod[:], AX.X, ALU.add)
                ssum = small.tile([128, 1], F32, tag="ssum")
                nc.vector.tensor_reduce(ssum[:], w8s[:], AX.X, ALU.add)
                rec = small.tile([128, 1], F32, tag="rec")
                nc.vector.reciprocal(rec[:], ssum[:])
                rec25 = small.tile([128, 1], F32, tag="rec25")
                nc.vector.tensor_scalar(rec25[:], rec[:], 2.5, None,
                                        op0=ALU.mult)
                wfin = small.tile([128, 8], F32, tag="wfin")
                nc.vector.tensor_scalar(wfin[:], w8[:], rec25[:], None,
                                        op0=ALU.mult)
                nc.sync.dma_start(o_w[b * 128:(b + 1) * 128], wfin[:])
                nc.sync.dma_start(o_idx[b * 128:(b + 1) * 128],
                                  i8[:].bitcast(I32))
    nc.compile()
    _cache["nc"] = nc
    return nc


def _prep(hidden_states, weight, e_score_correction_bias):
    x = np.asarray(hidden_states, dtype=np.float32)
    w = np.asarray(weight, dtype=np.float32)
    b = np.asarray(e_score_correction_bias, dtype=np.float32)

    xh = x.astype(np.float16)
    xl = ((x - xh.astype(np.float32)) * LSCALE).astype(np.float16)
    wh = w.astype(np.float16)
    wl = ((w - wh.astype(np.float32)) * LSCALE).astype(np.float16)

    # weights: [E, H] -> [NC, 128p, KC, E]
    def wtile(a):
        t = a.T.reshape(NC, KC, 128, E)           # [c, k, p, e]
        return np.ascontiguousarray(t.transpose(0, 2, 1, 3))

    wh_t, wl_t = wtile(wh), wtile(wl)

    # tokens: per core [TPC, H] -> [NB, NC, 128p, KC, 128t]
    def xtile(a, c):
        xs = a[c * TPC:(c + 1) * TPC]             # [1024, H]
        t = xs.T.reshape(NC, KC, 128, NB, 128)    # [c, k, p, b, t]
        return np.ascontiguousarray(t.transpose(3, 0, 2, 1, 4))

    in_maps = []
    for c in range(NCORES):
        in_maps.append({"xh": xtile(xh, c), "xl": xtile(xl, c),
                        "wh": wh_t, "wl": wl_t, "bias": b})
    return in_maps


def kernel(hidden_states, weight, e_score_correction_bias):
    nc = _build()
    in_maps = _prep(hidden_states, weight, e_score_correction_bias)
    global LAST_RESULTS
    br = run_bass_kernel_spmd(nc, in_maps, list(range(NCORES)), trace=TRACE)
    LAST_RESULTS = br
    res = br.results
    idx = np.concatenate([res[c]["o_idx"] for c in range(NCORES)], axis=0)
    wgt = np.concatenate([res[c]["o_w"] for c in range(NCORES)], axis=0)
    return idx.astype(np.int32), wgt.astype(np.float32)
